# revision 1
# baseline (speedup 1.0000x reference)
"""Trainium2 Bass kernel for nn_EstimatorQNNGen104 (dense tiny-MLP over 4.2M rows).

Strategy (pure data parallel over 8 NeuronCores):
  - Shard batch across cores: R_core = B/8 = 524288 rows/core.
  - Host packs each core's (R_core, 4) input into a feature-banded layout:
    supertile ST = 8192 rows = 16 groups x 512 cols; band f occupies
    partitions [16f, 16f+16) (+64 for the "hi" ST of a pair).
  - All matmuls are full-width (128,128) float32r (TF32-class, 1 cyc/row)
    with block-diagonal lhsT matrices built on the host; PSUM outputs always
    start at partition 0; narrow outputs stack into shared banks via
    zero-column lhsT padding + PSUM accumulation.
  - Activations (tanh/sigmoid) run on ScalarE with per-partition bias APs
    (all layer biases folded there); softmax(2) is computed as
    s0 = sigmoid(d), s1 = 1-s0 folded into the next layer's bias/weights.
  - Stage graph per supertile-pair:
      A: x -> samp_pre(4 bands) [bankA], conv_pre(1) [bankZ]
      tanhA -> samp_h;  B: samp_h -> d [bankZ]
      sigmoid(bankZ) -> sigc, s0   (bankZ holds 2 pairs: full 128 partitions)
      C: [x | sigc,s0] -> h1 (8 bands, 2 banks CA/CB); tanh -> h1
      D: h1 -> h2 (4 bands); tanh -> h2
      E: h2 -> out band, accumulated 4 pairs per bankE; + eb3 on VectorE;
      contiguous DMA out.
"""
import numpy as np
from contextlib import ExitStack

B_TOTAL = 4194304
N_CORES = 8
R_CORE = B_TOTAL // N_CORES        # 524288
G = 16
COLS = 512
ST_ROWS = G * COLS                  # 8192
N_ST = R_CORE // ST_ROWS            # 64
N_PAIRS = N_ST // 2                 # 32
N_W = N_PAIRS // 4                  # 8 windows

MAT_NAMES = ["A_samp", "A_conv_e", "A_conv_o", "B_e", "B_o",
             "C1_lo", "C1_hi", "C2_lo_e", "C2_lo_o", "C2_hi_e", "C2_hi_o",
             "D_lo", "D_hi", "E_0", "E_1", "E_2", "E_3"]
BIAS_NAMES = ["tanhA", "sigZ", "tanhC", "tanhD", "eb3"]


# ---------------- host-side weight/layout construction ----------------

def _band_block(W, n_in, n_out, col0=0, row0=0, mirror=True):
    """lhsT with rows row0+16f+g -> cols col0+16o+g weighted W[f,o].
    mirror=True replicates the [0,64) block into [64,128) (lo|hi halves)."""
    L = np.zeros((128, 128), np.float32)
    W = np.asarray(W, np.float32).reshape(n_in, n_out)
    for f in range(n_in):
        for o in range(n_out):
            w = W[f, o]
            if w == 0.0:
                continue
            for g in range(G):
                L[row0 + 16 * f + g, col0 + 16 * o + g] = w
    if mirror:
        L[64:, 64:] = L[:64, :64]
        L[64:, :64] = 0.0
    return L


def _build_weights(conv_w, conv_b, sW1, sb1, sW2, sb2,
                   eW1, eb1, eW2, eb2, eW3, eb3):
    mats = {}
    Wsamp = np.zeros((4, 4), np.float32)
    Wsamp[0:2, :] = sW1
    mats["A_samp"] = _band_block(Wsamp, 4, 4)
    cw = np.asarray(conv_w, np.float32).reshape(4, 1)
    mats["A_conv_e"] = _band_block(cw, 4, 1, col0=0)
    mats["A_conv_o"] = _band_block(cw, 4, 1, col0=32)
    dw = (sW2[:, 0] - sW2[:, 1]).reshape(4, 1)
    mats["B_e"] = _band_block(dw, 4, 1, col0=16)
    mats["B_o"] = _band_block(dw, 4, 1, col0=48)
    # C banks are per-supertile: bankCA = all 8 h1 bands of the LO supertile
    # (128 partitions), bankCB = HI supertile. One tanh bias vector for both.
    mats["C1_lo"] = _band_block(eW1[0:4, :], 4, 8, mirror=False)
    mats["C1_hi"] = _band_block(eW1[0:4, :], 4, 8, row0=64, mirror=False)
    W2 = np.stack([eW1[4], eW1[5] - eW1[6]], axis=0)  # (2,8)
    mats["C2_lo_e"] = _band_block(W2, 2, 8, row0=0, mirror=False)
    mats["C2_lo_o"] = _band_block(W2, 2, 8, row0=32, mirror=False)
    mats["C2_hi_e"] = _band_block(W2, 2, 8, row0=64, mirror=False)
    mats["C2_hi_o"] = _band_block(W2, 2, 8, row0=96, mirror=False)
    # D: contraction over all 8 h1 bands (128 partitions of one ST's bank);
    # h2_lo lands in cols [0,64), h2_hi in cols [64,128) of bankD.
    mats["D_lo"] = _band_block(eW2, 8, 4, mirror=False)
    mats["D_hi"] = _band_block(eW2, 8, 4, col0=64, mirror=False)
    for k in range(4):
        mats[f"E_{k}"] = _band_block(eW3.reshape(4, 1), 4, 1, col0=16 * k)

    def rep_half(vals4):
        v = np.repeat(np.asarray(vals4, np.float32), G)
        return np.concatenate([v, v])

    biases = {}
    biases["tanhA"] = rep_half(sb1)
    d_bias = np.float32(sb2[0] - sb2[1])
    biases["sigZ"] = rep_half([conv_b[0], d_bias, conv_b[0], d_bias])
    eb1_eff = (eb1 + eW1[6]).astype(np.float32)
    biases["tanhC"] = np.repeat(eb1_eff, G)  # 8 bands x 16 = 128
    biases["tanhD"] = rep_half(eb2)
    biases["eb3"] = np.full(128, np.float32(eb3[0]))

    Wpack = np.stack([mats[n] for n in MAT_NAMES], axis=1)  # (128, n_mats, 128)
    Wpack = np.ascontiguousarray(Wpack.reshape(128, len(MAT_NAMES) * 128))
    Bpack = np.ascontiguousarray(
        np.stack([biases[n] for n in BIAS_NAMES], axis=1))  # (128, 6)
    return Wpack, Bpack


def _pack_inputs(x):
    """x: (R_CORE, 4) -> (N_PAIRS, 128, 512) float32."""
    xv = x.reshape(N_W, 8, G, COLS, 4)       # [w, stidx, g, c, f]
    xv = xv.reshape(N_W, 2, 4, G, COLS, 4)   # [w, half, k, g, c, f]
    xv = xv.transpose(0, 2, 1, 5, 3, 4)      # [w, k, half, f, g, c]
    return np.ascontiguousarray(xv.reshape(N_PAIRS, 128, COLS), np.float32)


# ---------------- device program ----------------

_CACHED = {}


def _build_program(repeat=1):
    import concourse.bacc as bacc
    import concourse.tile as tile
    from concourse import mybir

    F32 = mybir.dt.float32
    F32R = mybir.dt.float32r
    AF = mybir.ActivationFunctionType

    nc = bacc.Bacc("TRN2", target_bir_lowering=False, debug=False)
    x_d = nc.dram_tensor("X", [N_PAIRS, 128, COLS], F32R, kind="ExternalInput")
    w_d = nc.dram_tensor("W", [128, len(MAT_NAMES) * 128], F32R,
                         kind="ExternalInput")
    b_d = nc.dram_tensor("BIAS", [128, len(BIAS_NAMES)], F32,
                         kind="ExternalInput")
    y_d = nc.dram_tensor("Y", [N_W, 128, COLS], F32, kind="ExternalOutput")

    M = {n: i for i, n in enumerate(MAT_NAMES)}
    BI = {n: i for i, n in enumerate(BIAS_NAMES)}

    with tile.TileContext(nc) as tc, ExitStack() as ctx:
        const = ctx.enter_context(tc.tile_pool(name="const", bufs=1))
        xp = ctx.enter_context(tc.tile_pool(name="xp", bufs=5))
        sampp = ctx.enter_context(tc.tile_pool(name="sampp", bufs=2))
        w4p = ctx.enter_context(tc.tile_pool(name="w4p", bufs=2))
        h1p = ctx.enter_context(tc.tile_pool(name="h1p", bufs=2))
        h2p = ctx.enter_context(tc.tile_pool(name="h2p", bufs=2))
        pA = ctx.enter_context(tc.tile_pool(name="pA", bufs=1, space="PSUM"))
        pZ = ctx.enter_context(tc.tile_pool(name="pZ", bufs=1, space="PSUM"))
        pC = ctx.enter_context(tc.tile_pool(name="pC", bufs=1, space="PSUM"))
        pD = ctx.enter_context(tc.tile_pool(name="pD", bufs=1, space="PSUM"))
        accp = ctx.enter_context(tc.tile_pool(name="accp", bufs=2))

        N_EARLY = 5  # A_samp .. B_o gate the pipeline head
        wt_early = const.tile([128, N_EARLY * 128], F32R)
        nc.sync.dma_start(out=wt_early[:], in_=w_d[:, 0:N_EARLY * 128])
        wt_late = const.tile([128, (len(MAT_NAMES) - N_EARLY) * 128], F32R)
        bt = const.tile([128, len(BIAS_NAMES)], F32)
        nc.sync.dma_start(out=bt[:], in_=b_d[:])
        # Warm the ACT table set (tanh/sigmoid) at t=0, overlapping DMA priming.
        warm = const.tile([128, 1], F32)
        nc.scalar.activation(warm[:], bt[:, 0:1], AF.Tanh)
        nc.scalar.activation(warm[:], bt[:, 0:1], AF.Sigmoid)

        def W(name):
            m = M[name]
            if m < N_EARLY:
                return wt_early[:, m * 128:(m + 1) * 128]
            m -= N_EARLY
            return wt_late[:, m * 128:(m + 1) * 128]

        def bias(name):
            return bt[:, BI[name]:BI[name] + 1]

        zt = None
        pa = None
        samp = None
        x2_hold = [None, None]
        for p_rep in range(N_PAIRS * repeat):
            p = p_rep % N_PAIRS
            k = p % 4
            w = p // 4
            eo = "e" if k % 2 == 0 else "o"
            h = (k % 2) * COLS  # free offset within paired (128,1024) banks

            x2 = xp.tile([128, COLS], F32R, tag="x2")
            nc.sync.dma_start(out=x2[:], in_=x_d[p])
            x2_hold[k % 2] = x2
            if p_rep == 1:
                # Deferred: the late weights (C/D/E) aren't needed until the
                # first C matmul; emitting here keeps the HWDGE FIFO from
                # stalling the first X tiles behind 1.2MB of weights.
                nc.sync.dma_start(out=wt_late[:], in_=w_d[:, N_EARLY * 128:])

            # stage A: two pairs share one (128,1024) bank -> one tanh op
            if k % 2 == 0:
                pa = pA.tile([128, 2 * COLS], F32, tag="pa")
            nc.tensor.matmul(pa[:, h:h + COLS], W("A_samp"), x2[:],
                             start=True, stop=True, skip_group_check=True)

            # bank Z: conv_pre + d for two consecutive pairs
            if k % 2 == 0:
                zt = pZ.tile([128, COLS], F32, tag="pz")
            nc.tensor.matmul(zt[:], W(f"A_conv_{eo}"), x2[:],
                             start=(k % 2 == 0), stop=False,
                             skip_group_check=True)

            if k % 2 == 1:
                samp = sampp.tile([128, 2 * COLS], F32R, tag="samp")
                nc.scalar.activation(samp[:], pa[:], AF.Tanh,
                                     bias=bias("tanhA"))
                nc.tensor.matmul(zt[:], W("B_e"), samp[:, 0:COLS],
                                 start=False, stop=False,
                                 skip_group_check=True)
                nc.tensor.matmul(zt[:], W("B_o"), samp[:, COLS:2 * COLS],
                                 start=False, stop=True,
                                 skip_group_check=True)
                w4 = w4p.tile([128, COLS], F32R, tag="w4")
                nc.scalar.activation(w4[:], zt[:], AF.Sigmoid,
                                     bias=bias("sigZ"))
                # stages C..E for both pairs of this Z window.
                # Both pairs' C outputs share one (128,2048) 4-bank tile ->
                # a single tanh op per group.
                pc = pC.tile([128, 4 * COLS], F32, tag="pc")
                for kk in (k - 1, k):
                    ee = "e" if kk % 2 == 0 else "o"
                    x2k = x2_hold[kk % 2]
                    q = (kk % 2) * 2 * COLS
                    nc.tensor.matmul(pc[:, q:q + COLS], W("C1_lo"), x2k[:],
                                     start=True, stop=False,
                                     skip_group_check=True)
                    nc.tensor.matmul(pc[:, q:q + COLS], W(f"C2_lo_{ee}"),
                                     w4[:], start=False, stop=True,
                                     skip_group_check=True)
                    nc.tensor.matmul(pc[:, q + COLS:q + 2 * COLS], W("C1_hi"),
                                     x2k[:], start=True, stop=False,
                                     skip_group_check=True)
                    nc.tensor.matmul(pc[:, q + COLS:q + 2 * COLS],
                                     W(f"C2_hi_{ee}"), w4[:],
                                     start=False, stop=True,
                                     skip_group_check=True)
                h1 = h1p.tile([128, 4 * COLS], F32R, tag="h1")
                nc.scalar.activation(h1[:], pc[:], AF.Tanh,
                                     bias=bias("tanhC"))
                for kk in (k - 1, k):
                    q = (kk % 2) * 2 * COLS
                    pd = pD.tile([128, COLS], F32, tag="pd")
                    nc.tensor.matmul(pd[:], W("D_lo"), h1[:, q:q + COLS],
                                     start=True, stop=False,
                                     skip_group_check=True)
                    nc.tensor.matmul(pd[:], W("D_hi"),
                                     h1[:, q + COLS:q + 2 * COLS],
                                     start=False, stop=True,
                                     skip_group_check=True)
                    h2 = h2p.tile([128, COLS], F32R, tag="h2")
                    nc.scalar.activation(h2[:], pd[:], AF.Tanh,
                                         bias=bias("tanhD"))
                    pe_t = pD.tile([128, COLS], F32, tag="pd")
                    nc.tensor.matmul(pe_t[:], W(f"E_{kk}"), h2[:],
                                     start=True, stop=True,
                                     skip_group_check=True)
                    if kk == 0:
                        acc = accp.tile([128, COLS], F32, tag="acc")
                        nc.vector.tensor_scalar_add(acc[:], pe_t[:],
                                                    bias("eb3"))
                    else:
                        nc.vector.tensor_add(acc[:], acc[:], pe_t[:])
                    if kk == 3:
                        nc.sync.dma_start(out=y_d[w], in_=acc[:])

    nc.compile()
    return nc


def kernel(**inputs):
    from concourse.bass_utils import run_bass_kernel_spmd

    inputs = {k: np.asarray(v, np.float32) for k, v in inputs.items()}
    x = inputs["inputs"]
    Wpack, Bpack = _build_weights(
        inputs["conv_w"], inputs["conv_b"], inputs["sW1"], inputs["sb1"],
        inputs["sW2"], inputs["sb2"], inputs["eW1"], inputs["eb1"],
        inputs["eW2"], inputs["eb2"], inputs["eW3"], inputs["eb3"])

    if "nc" not in _CACHED:
        _CACHED["nc"] = _build_program()
    nc = _CACHED["nc"]

    in_maps = []
    for c in range(N_CORES):
        xc = x[c * R_CORE:(c + 1) * R_CORE]
        in_maps.append({"X": _pack_inputs(xc), "W": Wpack, "BIAS": Bpack})

    res = run_bass_kernel_spmd(nc, in_maps, list(range(N_CORES)))
    out = np.concatenate(
        [res.results[c]["Y"].reshape(R_CORE, 1) for c in range(N_CORES)],
        axis=0)
    return out.astype(np.float32)



# revision 23
# speedup vs baseline: 1.0722x; 1.0722x over previous
"""Trainium2 Bass kernel for nn_EstimatorQNNGen104 (dense tiny-MLP over 4.2M rows).

Strategy (pure data parallel over 8 NeuronCores):
  - Shard batch across cores: R_core = B/8 = 524288 rows/core.
  - Host packs each core's (R_core, 4) input into a feature-banded layout:
    supertile ST = 8192 rows = 16 groups x 512 cols; band f occupies
    partitions [16f, 16f+16) (+64 for the "hi" ST of a pair).
  - All matmuls are full-width (128,128) float32r (TF32-class, 1 cyc/row)
    with block-diagonal lhsT matrices built on the host; PSUM outputs always
    start at partition 0; narrow outputs stack into shared banks via
    zero-column lhsT padding + PSUM accumulation.
  - Activations (tanh/sigmoid) run on ScalarE with per-partition bias APs
    (all layer biases folded there); softmax(2) is computed as
    s0 = sigmoid(d), s1 = 1-s0 folded into the next layer's bias/weights.
  - PSUM banks (8 total): pa(1) samp_pre per pair; zt(1) conv_pre+d per
    group of 2 pairs; pc(4) h1_pre per group; pd(1) h2_pre per pair;
    pe(1) window accumulator: the E matmuls of the 4 pairs of a window
    band-stack via PSUM accumulation, then one DVE add (+eb3) drains to
    SBUF and the output DMA ships it (DMA cannot read PSUM).
  - Emission is software-pipelined with explicit per-engine slot orders so
    the ScalarE (bottleneck engine ~79us busy) never waits:
      iter i: Act: tanhD(i-2,1) tanhA(i+1,p0) tanhA(i+1,p1) sig(i+1)
                   tanhC(i) tanhD(i-1,0)
              PE:  C(i)x8  E(i-2,1) A(i+1)x4 B(i+1)x2 D(i-1,0)x2
                   E(i-1,0) D(i-1,1)x2
"""
import numpy as np
from contextlib import ExitStack

B_TOTAL = 4194304
N_CORES = 8
R_CORE = B_TOTAL // N_CORES        # 524288
G = 16
COLS = 512
ST_ROWS = G * COLS                  # 8192
N_ST = R_CORE // ST_ROWS            # 64
N_PAIRS = N_ST // 2                 # 32 pairs (16384 rows each)
N_G = N_PAIRS // 2                  # 16 groups (2 pairs)
N_W = N_PAIRS // 4                  # 8 windows (4 pairs)

MAT_NAMES = ["A_samp", "A_conv_e", "A_conv_o", "B_e", "B_o",
             "C1_lo", "C1_hi", "C2_lo_e", "C2_lo_o", "C2_hi_e", "C2_hi_o",
             "D_lo", "D_hi", "E_0", "E_1", "E_2", "E_3"]
BIAS_NAMES = ["tanhA", "sigZ", "tanhC", "tanhD", "eb3"]


# ---------------- host-side weight/layout construction ----------------

def _band_block(W, n_in, n_out, col0=0, row0=0, mirror=True):
    """lhsT with rows row0+16f+g -> cols col0+16o+g weighted W[f,o].
    mirror=True replicates the [0,64) block into [64,128) (lo|hi halves)."""
    L = np.zeros((128, 128), np.float32)
    W = np.asarray(W, np.float32).reshape(n_in, n_out)
    for f in range(n_in):
        for o in range(n_out):
            w = W[f, o]
            if w == 0.0:
                continue
            for g in range(G):
                L[row0 + 16 * f + g, col0 + 16 * o + g] = w
    if mirror:
        L[64:, 64:] = L[:64, :64]
        L[64:, :64] = 0.0
    return L


def _build_weights(conv_w, conv_b, sW1, sb1, sW2, sb2,
                   eW1, eb1, eW2, eb2, eW3, eb3):
    mats = {}
    Wsamp = np.zeros((4, 4), np.float32)
    Wsamp[0:2, :] = sW1
    mats["A_samp"] = _band_block(Wsamp, 4, 4)
    cw = np.asarray(conv_w, np.float32).reshape(4, 1)
    mats["A_conv_e"] = _band_block(cw, 4, 1, col0=0)
    mats["A_conv_o"] = _band_block(cw, 4, 1, col0=32)
    dw = (sW2[:, 0] - sW2[:, 1]).reshape(4, 1)
    mats["B_e"] = _band_block(dw, 4, 1, col0=16)
    mats["B_o"] = _band_block(dw, 4, 1, col0=48)
    # C banks are per-supertile: bankCA = all 8 h1 bands of the LO supertile
    # (128 partitions), bankCB = HI supertile. One tanh bias vector for both.
    mats["C1_lo"] = _band_block(eW1[0:4, :], 4, 8, mirror=False)
    mats["C1_hi"] = _band_block(eW1[0:4, :], 4, 8, row0=64, mirror=False)
    W2 = np.stack([eW1[4], eW1[5] - eW1[6]], axis=0)  # (2,8)
    mats["C2_lo_e"] = _band_block(W2, 2, 8, row0=0, mirror=False)
    mats["C2_lo_o"] = _band_block(W2, 2, 8, row0=32, mirror=False)
    mats["C2_hi_e"] = _band_block(W2, 2, 8, row0=64, mirror=False)
    mats["C2_hi_o"] = _band_block(W2, 2, 8, row0=96, mirror=False)
    # D: contraction over all 8 h1 bands (128 partitions of one ST's bank);
    # h2_lo lands in cols [0,64), h2_hi in cols [64,128) of bankD.
    mats["D_lo"] = _band_block(eW2, 8, 4, mirror=False)
    mats["D_hi"] = _band_block(eW2, 8, 4, col0=64, mirror=False)
    for k in range(4):
        mats[f"E_{k}"] = _band_block(eW3.reshape(4, 1), 4, 1, col0=16 * k)

    def rep_half(vals4):
        v = np.repeat(np.asarray(vals4, np.float32), G)
        return np.concatenate([v, v])

    biases = {}
    biases["tanhA"] = rep_half(sb1)
    d_bias = np.float32(sb2[0] - sb2[1])
    biases["sigZ"] = rep_half([conv_b[0], d_bias, conv_b[0], d_bias])
    eb1_eff = (eb1 + eW1[6]).astype(np.float32)
    biases["tanhC"] = np.repeat(eb1_eff, G)  # 8 bands x 16 = 128
    biases["tanhD"] = rep_half(eb2)
    biases["eb3"] = np.full(128, np.float32(eb3[0]))

    Wpack = np.stack([mats[n] for n in MAT_NAMES], axis=1)  # (128, n_mats, 128)
    Wpack = np.ascontiguousarray(Wpack.reshape(128, len(MAT_NAMES) * 128))
    Bpack = np.ascontiguousarray(
        np.stack([biases[n] for n in BIAS_NAMES], axis=1))  # (128, 5)
    return Wpack, Bpack


def _pack_inputs(x):
    """x: (R_CORE, 4) -> (N_PAIRS, 128, 512) float32."""
    xv = x.reshape(N_W, 8, G, COLS, 4)       # [w, stidx, g, c, f]
    xv = xv.reshape(N_W, 2, 4, G, COLS, 4)   # [w, half, k, g, c, f]
    xv = xv.transpose(0, 2, 1, 5, 3, 4)      # [w, k, half, f, g, c]
    return np.ascontiguousarray(xv.reshape(N_PAIRS, 128, COLS), np.float32)


# ---------------- device program ----------------

_CACHED = {}


def _build_program():
    import concourse.bacc as bacc
    import concourse.tile as tile
    from concourse import mybir

    F32 = mybir.dt.float32
    F32R = mybir.dt.float32r
    AF = mybir.ActivationFunctionType

    nc = bacc.Bacc("TRN2", target_bir_lowering=False, debug=False)
    x_d = nc.dram_tensor("X", [N_PAIRS, 128, COLS], F32R, kind="ExternalInput")
    w_d = nc.dram_tensor("W", [128, len(MAT_NAMES) * 128], F32R,
                         kind="ExternalInput")
    b_d = nc.dram_tensor("BIAS", [128, len(BIAS_NAMES)], F32,
                         kind="ExternalInput")
    y_d = nc.dram_tensor("Y", [N_W, 128, COLS], F32, kind="ExternalOutput")

    M = {n: i for i, n in enumerate(MAT_NAMES)}
    BI = {n: i for i, n in enumerate(BIAS_NAMES)}

    with tile.TileContext(nc) as tc, ExitStack() as ctx:
        const = ctx.enter_context(tc.tile_pool(name="const", bufs=1))
        xp = ctx.enter_context(tc.tile_pool(name="xp", bufs=10))
        sampp = ctx.enter_context(tc.tile_pool(name="sampp", bufs=4))
        w4p = ctx.enter_context(tc.tile_pool(name="w4p", bufs=2))
        h1p = ctx.enter_context(tc.tile_pool(name="h1p", bufs=3))
        h2p = ctx.enter_context(tc.tile_pool(name="h2p", bufs=3))
        accp = ctx.enter_context(tc.tile_pool(name="accp", bufs=2))
        pA = ctx.enter_context(tc.tile_pool(name="pA", bufs=1, space="PSUM"))
        pZ = ctx.enter_context(tc.tile_pool(name="pZ", bufs=1, space="PSUM"))
        pC = ctx.enter_context(tc.tile_pool(name="pC", bufs=1, space="PSUM"))
        pD = ctx.enter_context(tc.tile_pool(name="pD", bufs=1, space="PSUM"))
        pE = ctx.enter_context(tc.tile_pool(name="pE", bufs=1, space="PSUM"))

        # --- prologue DMAs: bias first (tiny -> unblocks ACT warm-up),
        # then weights split into chunks interleaved with the first x tiles
        # so no single transfer stalls the pipeline head.
        bt = const.tile([128, len(BIAS_NAMES)], F32)
        # Warm the ACT tables at t=0. Sigmoid FIRST: the table-load pass then
        # picks the set that contains both sigmoid and tanh, so only one
        # 1.3us table load is ever paid (tanh-first would load a tanh-only
        # set and reload for sigmoid).
        warm = const.tile([128, 1], F32)
        nc.scalar.activation(warm[:], bt[:, 0:1], AF.Sigmoid)
        nc.scalar.activation(warm[:], bt[:, 0:1], AF.Tanh)

        wt_a = const.tile([128, 5 * 128], F32R)       # A_samp..B_o
        wt_c1 = const.tile([128, 4 * 128], F32R)      # C1_lo..C2_lo_o
        wt_c2 = const.tile([128, 4 * 128], F32R)      # C2_hi_e..D_hi
        wt_e = const.tile([128, 4 * 128], F32R)       # E_0..E_3

        # PE p-state pre-warm: dummy matmuls on a DMA-filled scratch tile keep
        # the PE busy from t~2.2 so the cost model's 3us ramp completes before
        # the first real matmul (which would otherwise run at 2-4x cycle).
        pewarm = const.tile([128, 256], F32R)
        nc.sync.dma_start(out=pewarm[:], in_=w_d[:, 0:256])
        warm_ps = pE.tile([128, COLS], F32, tag="pe")
        for _ in range(8):
            nc.tensor.matmul(warm_ps[:, 0:256], pewarm[:, 0:128], pewarm[:],
                             start=True, stop=True, skip_group_check=True)

        xt = {}  # pair index -> x tile

        def dma_x(g):
            for kk in (0, 1):
                p = 2 * g + kk
                t = xp.tile([128, COLS], F32R, tag="x2")
                nc.sync.dma_start(out=t[:], in_=x_d[p])
                xt[p] = t

        dma_x(0)
        nc.sync.dma_start(out=wt_a[:, 0:128], in_=w_d[:, 0:128])
        nc.sync.dma_start(out=bt[:], in_=b_d[:])
        nc.sync.dma_start(out=wt_a[:, 128:640], in_=w_d[:, 128:640])
        dma_x(1)
        nc.sync.dma_start(out=wt_c1[:], in_=w_d[:, 640:1152])
        dma_x(2)
        nc.sync.dma_start(out=wt_c2[:], in_=w_d[:, 1152:1664])
        nc.sync.dma_start(out=wt_e[:], in_=w_d[:, 1664:2176])

        def W(name):
            m = M[name]
            if m < 5:
                return wt_a[:, m * 128:(m + 1) * 128]
            if m < 9:
                return wt_c1[:, (m - 5) * 128:(m - 4) * 128]
            if m < 13:
                return wt_c2[:, (m - 9) * 128:(m - 8) * 128]
            return wt_e[:, (m - 13) * 128:(m - 12) * 128]

        def bias(name):
            return bt[:, BI[name]:BI[name] + 1]

        # --- pipeline state
        samp = {}   # pair -> samp tile (tanhA out)
        w4 = {}     # group -> sigmoid out tile
        h1 = {}     # group -> tanhC out tile (128, 2048)
        h2 = {}     # (group, kk) -> tanhD out tile
        pa_t = {}   # pair -> pa PSUM tile
        zt_t = {}   # group -> zt PSUM tile
        pc_t = {}   # group -> pc PSUM tile
        pd_t = {}   # (group, kk) -> pd PSUM tile
        pe_cur = [None]  # current window accumulator

        def emit_Asamp(g, kk):
            p = 2 * g + kk
            pa = pA.tile([128, COLS], F32, tag="pa")
            pa_t[p] = pa
            nc.tensor.matmul(pa[:], W("A_samp"), xt[p][:],
                             start=True, stop=True, skip_group_check=True)

        def emit_Aconv(g, kk):
            p = 2 * g + kk
            eo = "e" if kk == 0 else "o"
            if kk == 0:
                zt = pZ.tile([128, COLS], F32, tag="pz")
                zt_t[g] = zt
            nc.tensor.matmul(zt_t[g][:], W(f"A_conv_{eo}"), xt[p][:],
                             start=(kk == 0), stop=False,
                             skip_group_check=True)

        def emit_tanhA(g, kk):
            p = 2 * g + kk
            s = sampp.tile([128, COLS], F32R, tag="samp")
            samp[p] = s
            nc.scalar.activation(s[:], pa_t[p][:], AF.Tanh, bias=bias("tanhA"))
            del pa_t[p]

        def emit_B(g, kk):
            eo = "e" if kk == 0 else "o"
            nc.tensor.matmul(zt_t[g][:], W(f"B_{eo}"), samp[2 * g + kk][:],
                             start=False, stop=(kk == 1),
                             skip_group_check=True)

        def emit_sig(g):
            t = w4p.tile([128, COLS], F32R, tag="w4")
            w4[g] = t
            nc.scalar.activation(t[:], zt_t[g][:], AF.Sigmoid,
                                 bias=bias("sigZ"))
            del zt_t[g]

        def emit_C1(g, kk, half, alloc=False, del_x=False):
            if alloc:
                pc = pC.tile([128, 4 * COLS], F32, tag="pc")
                pc_t[g] = pc
            p = 2 * g + kk
            q = kk * 2 * COLS + (0 if half == "lo" else COLS)
            nc.tensor.matmul(pc_t[g][:, q:q + COLS], W(f"C1_{half}"),
                             xt[p][:], start=True, stop=False,
                             skip_group_check=True)
            if del_x:
                del xt[p]

        def emit_C2(g, kk, half):
            eo = "e" if kk == 0 else "o"
            q = kk * 2 * COLS + (0 if half == "lo" else COLS)
            nc.tensor.matmul(pc_t[g][:, q:q + COLS], W(f"C2_{half}_{eo}"),
                             w4[g][:], start=False, stop=True,
                             skip_group_check=True)

        def emit_tanhC(g):
            t = h1p.tile([128, 4 * COLS], F32R, tag="h1")
            h1[g] = t
            nc.scalar.activation(t[:], pc_t[g][:], AF.Tanh, bias=bias("tanhC"))
            del pc_t[g]
            if g >= 1:
                del w4[g - 1]

        def emit_D(g, kk, pool=None):
            q = kk * 2 * COLS
            if pool is None:
                pd = pD.tile([128, COLS], F32, tag="pd")
            else:
                pd = pool.tile([128, COLS], F32, tag="pa")
            pd_t[(g, kk)] = pd
            nc.tensor.matmul(pd[:], W("D_lo"), h1[g][:, q:q + COLS],
                             start=True, stop=False, skip_group_check=True)
            nc.tensor.matmul(pd[:], W("D_hi"), h1[g][:, q + COLS:q + 2 * COLS],
                             start=False, stop=True, skip_group_check=True)

        def emit_tanhD(g, kk):
            t = h2p.tile([128, COLS], F32R, tag="h2")
            h2[(g, kk)] = t
            nc.scalar.activation(t[:], pd_t[(g, kk)][:], AF.Tanh,
                                 bias=bias("tanhD"))
            del pd_t[(g, kk)]

        def emit_E(g, kk):
            kg = (2 * g + kk) % 4          # window-local pair index
            if kg == 0:
                pe = pE.tile([128, COLS], F32, tag="pe")
                pe_cur[0] = pe
            nc.tensor.matmul(pe_cur[0][:], W(f"E_{kg}"), h2[(g, kk)][:],
                             start=(kg == 0), stop=(kg == 3),
                             skip_group_check=True)
            del h2[(g, kk)]

        def emit_out(w):
            acc = accp.tile([128, COLS], F32, tag="acc")
            nc.vector.tensor_scalar_add(acc[:], pe_cur[0][:], bias("eb3"))
            nc.sync.dma_start(out=y_d[w], in_=acc[:])

        # --- software-pipelined main loop (cyclic schedule, period ~4.96us,
        # ScalarE-bound and gapless in steady state).
        # Act queue/period p: tanhD(p-3,0) tanhA(p,p0) tanhD(p-3,1)
        #                     tanhA(p,p1) tanhC(p-1) sig(p)
        # PE queue/period p:  Aconv(p,0) C1hi(p-1,1) C2loE(p-1) D(p-3,1)
        #                     A(p,1) C2hiE(p-1) C2loO(p-1) C2hiO(p-1)
        #                     B_e(p) B_o(p) D(p-2,0) E(p-3,*) Asamp(p+1,0)
        #                     C1(p)x3
        # The D stage lags 3 groups so its matmuls run during tanhC; C groups
        # split across the period boundary (C1 tail needs only x; C2 needs
        # the sigmoid output of the same period).
        emit_Asamp(0, 0)
        for p in range(0, N_G + 1):
            if 3 <= p + 3 < N_G:
                dma_x(p + 3)
            if 0 <= p - 3 < N_G:
                emit_tanhD(p - 3, 0)
            if 0 <= p < N_G:
                emit_Aconv(p, 0)
            if 0 <= p - 1 < N_G:
                emit_C1(p - 1, 1, "hi", del_x=True)
            if 0 <= p < N_G:
                emit_tanhA(p, 0)
            if 0 <= p - 1 < N_G:
                emit_C2(p - 1, 0, "lo")
            if 0 <= p - 3 < N_G:
                emit_D(p - 3, 1)
                emit_tanhD(p - 3, 1)
            if 0 <= p < N_G:
                emit_Asamp(p, 1)
                emit_Aconv(p, 1)
                emit_tanhA(p, 1)
            if 0 <= p - 1 < N_G:
                emit_C2(p - 1, 0, "hi")
                emit_C2(p - 1, 1, "lo")
                emit_C2(p - 1, 1, "hi")
                emit_tanhC(p - 1)
            if 0 <= p < N_G:
                emit_B(p, 0)
                emit_B(p, 1)
            if 0 <= p - 2 < N_G:
                emit_D(p - 2, 0)
            if 0 <= p - 3 < N_G:
                emit_E(p - 3, 0)
                emit_E(p - 3, 1)
                if (p - 3) % 2 == 1:
                    emit_out((p - 3) // 2)
            if 0 <= p < N_G:
                emit_sig(p)
            if 0 <= p + 1 < N_G:
                emit_Asamp(p + 1, 0)
            if 0 <= p < N_G:
                emit_C1(p, 0, "lo", alloc=True)
                emit_C1(p, 0, "hi", del_x=True)
                emit_C1(p, 1, "lo")

        # --- eager drain of the last two groups: alternate D tiles between
        # the pD bank and the now-idle pA bank so the final four tanhDs run
        # back-to-back on ScalarE instead of serializing through one bank.
        gl = N_G - 2
        emit_D(gl, 1, pool=pA)
        emit_tanhD(gl, 0)
        emit_E(gl, 0)
        emit_tanhD(gl, 1)
        emit_D(gl + 1, 0)
        emit_E(gl, 1)
        emit_tanhD(gl + 1, 0)
        emit_D(gl + 1, 1, pool=pA)
        emit_E(gl + 1, 0)
        emit_tanhD(gl + 1, 1)
        emit_E(gl + 1, 1)
        emit_out(N_W - 1)

    nc.compile()
    return nc


def kernel(**inputs):
    from concourse.bass_utils import run_bass_kernel_spmd

    inputs = {k: np.asarray(v, np.float32) for k, v in inputs.items()}
    x = inputs["inputs"]
    Wpack, Bpack = _build_weights(
        inputs["conv_w"], inputs["conv_b"], inputs["sW1"], inputs["sb1"],
        inputs["sW2"], inputs["sb2"], inputs["eW1"], inputs["eb1"],
        inputs["eW2"], inputs["eb2"], inputs["eW3"], inputs["eb3"])

    if "nc" not in _CACHED:
        _CACHED["nc"] = _build_program()
    nc = _CACHED["nc"]

    in_maps = []
    for c in range(N_CORES):
        xc = x[c * R_CORE:(c + 1) * R_CORE]
        in_maps.append({"X": _pack_inputs(xc), "W": Wpack, "BIAS": Bpack})

    res = run_bass_kernel_spmd(nc, in_maps, list(range(N_CORES)))
    out = np.concatenate(
        [res.results[c]["Y"].reshape(R_CORE, 1) for c in range(N_CORES)],
        axis=0)
    return out.astype(np.float32)
